# revision 20
# baseline (speedup 1.0000x reference)
"""Causal self-attention TRN2 kernel (v3).

B=4, T=2048, C=1024, H=16 heads, D=64. 8 NeuronCores: core = b*2 + g
(b = batch 0..3, g = head-group 0..1, 8 heads each). Tensor-parallel over
heads within a batch; host sums the two partial proj outputs per batch.

v3 structure: attention runs ci-outer / pr-inner. Per (pr, ci) sweep, each
jt-step issues S^T matmuls for the head pair (A rows 0-63, B rows 64-127)
into a 2-bank PSUM slab [128, 1024] drained by ONE exp ACTIVATE (N=1024),
with the causal mask DVE-multiplied on diagonal slabs. PV (K=128, M=65,
ones-column denominator) lags 4 jt-steps inside the same sweep. The
ScalarE is the sweep pacer (1147ns/slab vs ~970ns of mandatory PE work),
so a budget-based "pump" splices filler chains -- qkv projections for
other head pairs, v-aug tiles, and output proj chunks -- into each step to
keep the PE saturated and HAM warm. Chunks normalize per (pr, ci) via a
DMA-broadcast reciprocal fused into the accumulator drain; proj(ci) chains
are queued right after the last pair's chunk ci drains.
"""

import numpy as np
import ml_dtypes

B, T, C = 4, 2048, 1024
H, D = 16, 64
HPC = 8          # heads per core
P = 128
TCH = 512        # i-chunk (query) size
NCH = T // TCH   # 4
NJT = T // P     # 16 key tiles
CT = C // P      # 8 contraction tiles
VW = HPC * (D + 1)  # 520: augmented v width

_CACHE = {}


def _build_nc():
    import concourse.bass as bass
    import concourse.mybir as mybir
    import concourse.tile as tile
    from concourse import bacc
    from contextlib import ExitStack

    bf16 = mybir.dt.bfloat16
    fp32 = mybir.dt.float32
    AF = mybir.ActivationFunctionType

    nc = bacc.Bacc()
    xT_d = nc.dram_tensor("xT", [C, T], bf16, kind="ExternalInput")
    wq_d = nc.dram_tensor("wqT", [C, 512], bf16, kind="ExternalInput")
    wk_d = nc.dram_tensor("wkT", [C, 512], bf16, kind="ExternalInput")
    wv_d = nc.dram_tensor("wvT", [C, VW], bf16, kind="ExternalInput")
    bq_d = nc.dram_tensor("bq", [512, 1], fp32, kind="ExternalInput")
    bk_d = nc.dram_tensor("bk", [512, 1], fp32, kind="ExternalInput")
    bv_d = nc.dram_tensor("bv", [1, VW], bf16, kind="ExternalInput")
    mask_d = nc.dram_tensor("masks", [P, 2 * P], bf16, kind="ExternalInput")
    pw_d = nc.dram_tensor("pwT", [512, C], bf16, kind="ExternalInput")
    yT_d = nc.dram_tensor("yT", [C, T], fp32, kind="ExternalOutput")
    den_dram = nc.dram_tensor("den_scratch", [8, T], fp32)
    den2_dram = nc.dram_tensor("den2_scratch", [8, T], fp32)

    def bcast_part(ap, n):
        # replicate a [1, F] AP across n partitions (step-0 partition dim)
        return bass.AP(tensor=ap.tensor, offset=ap.offset,
                       ap=[[0, n]] + list(ap.ap[1:]))

    with ExitStack() as ctx:
        tc = ctx.enter_context(tile.TileContext(nc))
        consts = ctx.enter_context(tc.tile_pool(name="consts", bufs=1))
        xt_p = ctx.enter_context(tc.tile_pool(name="xt", bufs=1))
        vaug_p = ctx.enter_context(tc.tile_pool(name="vaug", bufs=1))
        qk_p = ctx.enter_context(tc.tile_pool(name="qk", bufs=4))
        pt_p = ctx.enter_context(tc.tile_pool(name="pt", bufs=10))
        ot_p = ctx.enter_context(tc.tile_pool(name="ot", bufs=1))
        rec_p = ctx.enter_context(tc.tile_pool(name="rec", bufs=2))
        st_p = ctx.enter_context(tc.tile_pool(name="st", bufs=4))
        ys_p = ctx.enter_context(tc.tile_pool(name="ys", bufs=3))
        ps_p = ctx.enter_context(tc.tile_pool(name="ps", bufs=1, space="PSUM"))

        # ---- constant loads, ordered so compute starts ASAP:
        # wv + x quarter 0 enable v(0..3); wq/wk enable qk(0,0).
        wq_t = consts.tile([P, CT, 512], bf16, tag="wq")
        wk_t = consts.tile([P, CT, 512], bf16, tag="wk")
        wv_t = consts.tile([P, CT, VW], bf16, tag="wv")
        xt = xt_p.tile([P, CT, T], bf16, tag="xt")
        for c in range(CT):
            nc.sync.dma_start(out=wv_t[:, c, :], in_=wv_d[c * P:(c + 1) * P, :])
        for c in range(CT):
            nc.sync.dma_start(out=xt[:, c, 0:TCH],
                              in_=xT_d[c * P:(c + 1) * P, 0:TCH])
        bv_t = consts.tile([P, VW], bf16, tag="bv")
        nc.sync.dma_start(out=bv_t, in_=bcast_part(bv_d[0:1, :], P))
        for c in range(CT):
            nc.sync.dma_start(out=wq_t[:, c, :], in_=wq_d[c * P:(c + 1) * P, :])
            nc.sync.dma_start(out=wk_t[:, c, :], in_=wk_d[c * P:(c + 1) * P, :])
        bq_t = consts.tile([P, 4, 1], fp32, tag="bq")
        bk_t = consts.tile([P, 4, 1], fp32, tag="bk")
        nc.sync.dma_start(out=bq_t, in_=bq_d.ap().rearrange("(a p) o -> p a o", p=P))
        nc.sync.dma_start(out=bk_t, in_=bk_d.ap().rearrange("(a p) o -> p a o", p=P))
        for q in range(1, 4):
            for c in range(CT):
                nc.sync.dma_start(
                    out=xt[:, c, q * TCH:(q + 1) * TCH],
                    in_=xT_d[c * P:(c + 1) * P, q * TCH:(q + 1) * TCH])
        mk_t = consts.tile([P, 2, P], bf16, tag="mk")
        nc.sync.dma_start(out=mk_t, in_=mask_d[:, :])
        pw_t = consts.tile([P, 4, C], bf16, tag="pw")
        for m in range(4):
            nc.sync.dma_start(out=pw_t[:, m, :], in_=pw_d[m * P:(m + 1) * P, :])

        vaug = vaug_p.tile([P, NJT, VW], bf16, tag="vaug")
        ot = ot_p.tile([P, 4, T], bf16, tag="ot")
        HLF = VW // 2  # 260
        qts = [qk_p.tile([P, T], bf16, tag="qt", name=f"qt{pr}")
               for pr in range(4)]
        kts = [qk_p.tile([P, T], bf16, tag="kt", name=f"kt{pr}")
               for pr in range(4)]

        # ---- filler chains: generators yielding after ~2 matmuls ----
        def v_chain(jt):
            ps0 = ps_p.tile([P, HLF], fp32, tag="misc", bufs=2)
            ps1 = ps_p.tile([P, HLF], fp32, tag="misc", bufs=2)
            for c in range(CT):
                lw = xt[:, c, jt * P:(jt + 1) * P]
                nc.tensor.matmul(ps0, lw, wv_t[:, c, 0:HLF],
                                 start=(c == 0), stop=(c == CT - 1))
                nc.tensor.matmul(ps1, lw, wv_t[:, c, HLF:VW],
                                 start=(c == 0), stop=(c == CT - 1))
                yield 230
            nc.vector.tensor_add(vaug[:, jt, 0:HLF], ps0, bv_t[:, 0:HLF])
            nc.vector.tensor_add(vaug[:, jt, HLF:VW], ps1, bv_t[:, HLF:VW])

        def qk_chain(pr, tch):
            psq = ps_p.tile([P, TCH], fp32, tag="misc", bufs=2)
            psk = ps_p.tile([P, TCH], fp32, tag="misc", bufs=2)
            for c in range(CT):
                rx = xt[:, c, tch * TCH:(tch + 1) * TCH]
                nc.tensor.matmul(psq, wq_t[:, c, pr * P:(pr + 1) * P], rx,
                                 start=(c == 0), stop=(c == CT - 1))
                nc.tensor.matmul(psk, wk_t[:, c, pr * P:(pr + 1) * P], rx,
                                 start=(c == 0), stop=(c == CT - 1))
                yield 432
            nc.vector.tensor_scalar_add(qts[pr][:, tch * TCH:(tch + 1) * TCH],
                                        psq, bq_t[:, pr, :])
            nc.vector.tensor_scalar_add(kts[pr][:, tch * TCH:(tch + 1) * TCH],
                                        psk, bk_t[:, pr, :])

        def proj_chain(ci):
            for n in range(CT):
                ps_y = ps_p.tile([P, TCH], fp32, tag="misc", bufs=2)
                for m in range(4):
                    nc.tensor.matmul(ps_y, pw_t[:, m, n * P:(n + 1) * P],
                                     ot[:, m, ci * TCH:(ci + 1) * TCH],
                                     start=(m == 0), stop=(m == 3))
                ys = ys_p.tile([P, TCH], fp32, tag="ys")
                nc.scalar.copy(ys, ps_y)
                nc.sync.dma_start(out=yT_d[n * P:(n + 1) * P,
                                           ci * TCH:(ci + 1) * TCH], in_=ys)
                yield 864

        filler_q = []
        active = [None]
        # virtual clocks (ns of emitted work): PE stream position vs the
        # ACT engine's slab-drain position.  Pump fillers only while the PE
        # stream is behind the ACT clock, so fillers last the whole
        # ACT-paced attention phase instead of being eaten early.
        clk = {"pe": 0.0, "act": 0.0}

        def pump(target):
            while clk["pe"] < target:
                if active[0] is None:
                    if not filler_q:
                        return
                    active[0] = filler_q.pop(0)
                try:
                    clk["pe"] += next(active[0])
                except StopIteration:
                    active[0] = None

        def run_chain(g):
            for c in g:
                clk["pe"] += c

        # ---- attention pieces ----
        def emit_pv_pair(pr, accA, accB, jt, pt2, nlo, st_, sp):
            for sub, acc in ((0, accA), (1, accB)):
                h = 2 * pr + sub
                nc.tensor.matmul(acc[0:65, nlo:TCH],
                                 vaug[:, jt, h * 65:h * 65 + 65],
                                 pt2[:, sub, nlo:TCH],
                                 start=st_, stop=sp)

        def drain_chunk(pr, ci, accA, accB):
            # stage den rows to SBUF (same partition), DMA out, reciprocal
            # via repack, broadcast back, normalize fused into the ot stage
            # copy
            for sub, acc in ((0, accA), (1, accB)):
                sd = st_p.tile([65, TCH], fp32, tag="std", bufs=2,
                               name=f"std{pr}_{ci}_{sub}")
                nc.vector.tensor_copy(sd[64:65, :], acc[64:65, :])
                nc.sync.dma_start(
                    out=den_dram[2 * pr + sub:2 * pr + sub + 1,
                                 ci * TCH:(ci + 1) * TCH],
                    in_=sd[64:65, :])

            def rpk(dram, h):
                base = dram.ap()
                return bass.AP(tensor=base.tensor,
                               offset=base.offset + h * T + ci * TCH,
                               ap=[[8, 64], [1, 8]])
            dt2 = st_p.tile([64, 2, 8], fp32, tag="dt", bufs=2,
                            name=f"dt{pr}_{ci}")
            for sub in range(2):
                nc.sync.dma_start(out=dt2[:, sub, :],
                                  in_=rpk(den_dram, 2 * pr + sub))
            nc.vector.reciprocal(dt2, dt2)
            for sub in range(2):
                nc.sync.dma_start(out=rpk(den2_dram, 2 * pr + sub),
                                  in_=dt2[:, sub, :])
            rr = rec_p.tile([64, 2, TCH], fp32, tag="rec", name=f"rr{pr}_{ci}")
            for sub in range(2):
                nc.sync.dma_start(
                    out=rr[:, sub, :],
                    in_=bcast_part(
                        den2_dram[2 * pr + sub:2 * pr + sub + 1,
                                  ci * TCH:(ci + 1) * TCH], 64))
            for sub, acc in ((0, accA), (1, accB)):
                stage = st_p.tile([64, TCH], bf16, tag="st", bufs=4,
                                  name=f"st{pr}_{ci}_{sub}")
                nc.vector.tensor_mul(stage, acc[0:64, :], rr[:, sub, :])
                nc.sync.dma_start(
                    out=ot[sub * 64:sub * 64 + 64, pr,
                           ci * TCH:(ci + 1) * TCH],
                    in_=stage)

        def sweep(pr, ci):
            qt, kt = qts[pr], kts[pr]
            njt = 4 * ci + 4
            accA = ps_p.tile([P, TCH], fp32, tag="acc", bufs=2)
            accB = ps_p.tile([P, TCH], fp32, tag="acc", bufs=2)
            pvq = []  # (jt, pt2) awaiting PV, lag 4 steps
            npv = 0

            def pop_pv(force=False):
                nonlocal npv
                while pvq and (force or len(pvq) > 4):
                    jt2, pt2, nlo2 = pvq.pop(0)
                    emit_pv_pair(pr, accA, accB, jt2, pt2, nlo2,
                                 npv == 0, npv == njt - 1)
                    npv += 1
                    clk["pe"] += 452 - nlo2 * 0.84
                    if force:
                        pump(clk["act"])

            for jt in range(njt):
                # quadrant-packed S^T: 4 concurrent (K=64, M=64) tiles --
                # (head, key-half) x (rows 0/64, psum partitions 0/64).
                # Diagonal tiles (r >= 0) only compute the valid causal
                # band: columns below 128*r are fully masked, so S, exp,
                # mask-mul and PV all skip them; the mask shrinks to a
                # fixed [128, 128] triangle on the band.
                r = jt - 4 * ci
                nlo = 128 * r if r > 0 else 0
                nv = TCH - nlo
                slab = ps_p.tile([P, 2, TCH], fp32, tag="slab", bufs=2)
                for sub in range(2):
                    rows = slice(sub * 64, sub * 64 + 64)
                    rq = qt[rows, ci * TCH + nlo:(ci + 1) * TCH]
                    for kh in range(2):
                        nc.tensor.matmul(
                            slab[kh * 64:kh * 64 + 64, sub, nlo:TCH],
                            kt[rows, jt * P + kh * 64:jt * P + kh * 64 + 64],
                            rq, start=True, stop=True)
                clk["pe"] += 350 - 106 * (r if r > 0 else 0)
                clk["act"] = max(clk["act"], clk["pe"]) \
                    + (2 * nv + 352) / 1.2
                pt2 = pt_p.tile([P, 2, TCH], bf16, tag="pt")
                nc.scalar.activation(pt2[:, :, nlo:TCH], slab[:, :, nlo:TCH],
                                     AF.Exp, scale=float(D) ** -0.5)
                if r >= 0:
                    nc.vector.tensor_mul(pt2[:, :, nlo:nlo + P],
                                         pt2[:, :, nlo:nlo + P], mk_t)
                pvq.append((jt, pt2, nlo))
                pop_pv()
                pump(clk["act"] - 1147)
            pop_pv(force=True)
            drain_chunk(pr, ci, accA, accB)

        # ---- pre-loop: v(0..3), qk(0,0), qk(1,0) run directly ----
        for jt in range(4):
            run_chain(v_chain(jt))
        run_chain(qk_chain(0, 0))
        run_chain(qk_chain(1, 0))

        # named chains so sweeps can force-complete their dependencies
        chains = {}
        for pr_ in range(4):
            for tch_ in range(NCH):
                if (pr_, tch_) not in ((0, 0), (1, 0)):
                    chains[("qk", pr_, tch_)] = qk_chain(pr_, tch_)
        for jt_ in range(4, NJT):
            chains[("v", jt_)] = v_chain(jt_)

        def force(key):
            g = chains.get(key)
            if g is not None:
                if active[0] is g:
                    active[0] = None
                else:
                    try:
                        filler_q.remove(g)
                    except ValueError:
                        return  # already fully drained
                run_chain(g)
                del chains[key]

        filler_q.extend([
            chains[("qk", 2, 0)], chains[("qk", 3, 0)],
            chains[("qk", 0, 1)], chains[("qk", 1, 1)],
            chains[("v", 4)], chains[("qk", 2, 1)], chains[("v", 5)],
            chains[("qk", 3, 1)], chains[("v", 6)],
            chains[("qk", 0, 2)], chains[("v", 7)],
            chains[("qk", 1, 2)], chains[("qk", 2, 2)],
            chains[("v", 8)], chains[("v", 9)],
            chains[("qk", 3, 2)], chains[("qk", 0, 3)],
            chains[("v", 10)], chains[("v", 11)],
            chains[("qk", 1, 3)], chains[("qk", 2, 3)],
            chains[("v", 12)], chains[("v", 13)],
            chains[("qk", 3, 3)], chains[("v", 14)], chains[("v", 15)],
        ])

        # force each sweep's dependencies ONE sweep ahead, so the forced
        # chains' DVE drains land before the consuming S/PV matmuls issue
        seq = [(ci, pr) for ci in range(NCH) for pr in range(4)]
        for idx, (ci, pr) in enumerate(seq):
            if idx + 1 < len(seq):
                nci, npr = seq[idx + 1]
                force(("qk", npr, nci))
                if nci != ci:
                    for jt_ in range(4 * nci, 4 * nci + 4):
                        force(("v", jt_))
            sweep(pr, ci)
            if pr == 3:
                filler_q.append(proj_chain(ci))
        # drain remaining fillers (tail proj chunks)
        while filler_q or active[0] is not None:
            pump(clk["pe"] + 100000)
    if not nc.is_finalized():
        nc.finalize()
    return nc


def _prep_inputs(x, qkv_w, qkv_b, proj_w):
    bf = ml_dtypes.bfloat16
    per_core = []
    wq, wk, wv = qkv_w[0:C], qkv_w[C:2 * C], qkv_w[2 * C:3 * C]
    bq, bk, bv = qkv_b[0:C], qkv_b[C:2 * C], qkv_b[2 * C:3 * C]
    # triangular causal band mask, duplicated for the [A|B] slab halves:
    # m[j, i] = 1 if j <= i  (band-local indices)
    jj = np.arange(P)[:, None]
    ii = np.arange(P)[None, :]
    mk1 = (jj <= ii).astype(bf)
    masks = np.concatenate([mk1, mk1], axis=1)
    xTs = [np.ascontiguousarray(x[b].T).astype(bf) for b in range(B)]
    for b in range(B):
        for g in range(2):
            hs = slice(g * 512, (g + 1) * 512)
            wvT_aug = np.zeros((C, VW), np.float32)
            bv_aug = np.zeros((1, VW), np.float32)
            for h in range(HPC):
                wvT_aug[:, h * 65:h * 65 + 64] = wv[hs][h * 64:(h + 1) * 64].T
                bv_aug[0, h * 65:h * 65 + 64] = bv[hs][h * 64:(h + 1) * 64]
                bv_aug[0, h * 65 + 64] = 1.0
            per_core.append({
                "xT": xTs[b],
                "wqT": np.ascontiguousarray(wq[hs].T).astype(bf),
                "wkT": np.ascontiguousarray(wk[hs].T).astype(bf),
                "wvT": wvT_aug.astype(bf),
                "bq": bq[hs].reshape(512, 1).astype(np.float32),
                "bk": bk[hs].reshape(512, 1).astype(np.float32),
                "bv": bv_aug.astype(bf),
                "masks": masks,
                "pwT": np.ascontiguousarray(proj_w[:, hs].T).astype(bf),
            })
    return per_core


def kernel(x, qkv_w, qkv_b, proj_w, proj_b, _trace=False):
    from concourse.bass_utils import run_bass_kernel_spmd

    x = np.asarray(x, np.float32)
    qkv_w = np.asarray(qkv_w, np.float32)
    qkv_b = np.asarray(qkv_b, np.float32)
    proj_w = np.asarray(proj_w, np.float32)
    proj_b = np.asarray(proj_b, np.float32)

    if "nc" not in _CACHE:
        _CACHE["nc"] = _build_nc()
    nc = _CACHE["nc"]
    in_maps = _prep_inputs(x, qkv_w, qkv_b, proj_w)
    res = run_bass_kernel_spmd(nc, in_maps, core_ids=list(range(8)),
                               trace=_trace)
    _CACHE["last_result"] = res
    y = np.empty((B, T, C), np.float32)
    for b in range(B):
        acc = res.results[2 * b]["yT"] + res.results[2 * b + 1]["yT"]
        y[b] = acc.T + proj_b
    return y
